# revision 54
# baseline (speedup 1.0000x reference)
"""MHA forward (B=4, S=1024, HID=768, NH=12, D=64) on 8 NeuronCores.

Sharding: core c -> batch b = c//2, head half = c%2 (6 heads each).
All on-device tensors live transposed ([feature, seq]) so every matmul
contraction dim lands on partitions with no on-device transposes:
  QT/KT [dh, s] <- WxT-slice^T @ hiddenT;  V [s, dh] <- hiddenT^T @ WvT
  ST [k, q] <- KT-slice^T @ QT-slice       (scores, no max-subtraction:
                                            |scores/8| < ~2 for this input dist)
  expS = exp(ST/8) via ACT (PSUM -> SBUF, bf16)
  ctxT [dh, q] <- V_aug^T @ expS, where V_aug has a ones column per head
                  -> row 64 of the PSUM tile is the softmax denominator
  probsT = expS * bcast(1/denom)  (bcast via GPSIMD partition_broadcast)
  outT [ho, s] <- WoT-slice^T @ ctxT
Host: converts weights/hidden to bf16, transposes probsT back (stored bf16,
upcast on host), sums the two per-batch outT partials, adds bo. PE runs bf16
with fp32 PSUM accumulation; heads are software-pipelined so the normalize
chain of head h overlaps head h+1 scores on DVE/Pool/DMA.
"""

import numpy as np

B, S, HID, NH, D = 4, 1024, 768, 12, 64
HPC = NH // 2          # heads per core = 6
DHC = HPC * D          # dh per core = 384
VW = D + 1             # V block width incl. ones column = 65

_CACHED_NC = None


def _build_nc():
    from contextlib import ExitStack

    import concourse.bass as bass
    import concourse.mybir as mybir
    import concourse.tile as tile
    from concourse import bacc

    f32 = mybir.dt.float32
    bf16 = mybir.dt.bfloat16
    Exp = mybir.ActivationFunctionType.Exp

    nc = bacc.Bacc("TRN2", target_bir_lowering=False, debug=False, num_devices=8)
    hT = nc.dram_tensor("hT", [HID + 1, S], bf16, kind="ExternalInput")
    wqT = nc.dram_tensor("wqT", [HID, DHC], bf16, kind="ExternalInput")
    wkT = nc.dram_tensor("wkT", [HID, DHC], bf16, kind="ExternalInput")
    wvT = nc.dram_tensor("wvT", [HID + 1, DHC], bf16, kind="ExternalInput")
    woT = nc.dram_tensor("woT", [DHC, HID], bf16, kind="ExternalInput")
    bqT = nc.dram_tensor("bqT", [DHC, 1], f32, kind="ExternalInput")
    bkT = nc.dram_tensor("bkT", [DHC, 1], f32, kind="ExternalInput")
    probT = nc.dram_tensor("probT", [HPC, S, S], bf16, kind="ExternalOutput")
    outT = nc.dram_tensor("outT", [HID, S], bf16, kind="ExternalOutput")

    NKB = HID // 128     # 6 contraction blocks for projections
    NPT = DHC // 128     # 3 partition tiles for QT/KT/ctxT
    NST = S // 128       # 8 seq tiles
    NQB = S // 512       # 2 moving blocks

    with (
        nc.allow_low_precision(reason="bf16 matmul staging; fp32 accumulate"),
        tile.TileContext(nc) as tc,
        ExitStack() as ctx,
    ):
        const = ctx.enter_context(tc.tile_pool(name="const", bufs=1))
        wpool = ctx.enter_context(tc.tile_pool(name="w", bufs=1))
        actp = ctx.enter_context(tc.tile_pool(name="acts", bufs=1))

        # ---- loads (consolidated: one DMA per tensor via rearranged APs) ----
        hT_sb = wpool.tile([128, NKB * S], bf16, tag="hT", name="hT_sb")
        wq_sb = wpool.tile([128, NKB * DHC], bf16, tag="wq", name="wq_sb")
        wk_sb = wpool.tile([128, NKB * DHC], bf16, tag="wk", name="wk_sb")
        wv_sb = wpool.tile([128, NKB * DHC], bf16, tag="wv", name="wv_sb")
        wo_sb = wpool.tile([128, NPT * HID], bf16, tag="wo", name="wo_sb")
        hT_one = wpool.tile([1, S], bf16, tag="hT_one")
        wv_one = wpool.tile([1, DHC], bf16, tag="wv_one")
        bq_sb = wpool.tile([128, NPT], f32, tag="bq", name="bq_sb")
        bk_sb = wpool.tile([128, NPT], f32, tag="bk", name="bk_sb")

        wq3 = wq_sb[:].rearrange("p (t d) -> p t d", t=NKB)
        wk3 = wk_sb[:].rearrange("p (t d) -> p t d", t=NKB)
        hT3 = hT_sb[:].rearrange("p (t s) -> p t s", t=NKB)
        wqT3 = wqT[:].rearrange("(t p) d -> p t d", p=128)
        wkT3 = wkT[:].rearrange("(t p) d -> p t d", p=128)
        hTT3 = hT[0:HID, :].rearrange("(t p) s -> p t s", p=128)
        for c in range(3):
            ksl = slice(2 * c, 2 * c + 2)
            nc.sync.dma_start(wq3[:, ksl], wqT3[:, ksl])
            nc.sync.dma_start(wk3[:, ksl], wkT3[:, ksl])
            nc.sync.dma_start(hT3[:, ksl], hTT3[:, ksl])
        nc.sync.dma_start(
            bq_sb[:].rearrange("p (t o) -> p t o", o=1),
            bqT[:].rearrange("(t p) o -> p t o", p=128),
        )
        nc.sync.dma_start(
            bk_sb[:].rearrange("p (t o) -> p t o", o=1),
            bkT[:].rearrange("(t p) o -> p t o", p=128),
        )
        nc.sync.dma_start(hT_one[:], hT[HID:HID + 1, :])
        nc.sync.dma_start(
            wv_sb[:].rearrange("p (t d) -> p t d", t=NKB),
            wvT[0:HID, :].rearrange("(t p) d -> p t d", p=128),
        )
        nc.sync.dma_start(wv_one[:], wvT[HID:HID + 1, :])
        nc.sync.dma_start(
            wo_sb[:].rearrange("p (t d) -> p t d", t=NPT),
            woT[:].rearrange("(t p) d -> p t d", p=128),
        )

        ones_row = const.tile([1, 128], bf16, tag="ones_row")
        nc.vector.memset(ones_row[:], 1.0)

        qt_sb = [actp.tile([128, S], bf16, tag=f"qt{i}", name=f"qt{i}") for i in range(NPT)]
        kt_sb = [actp.tile([128, S], bf16, tag=f"kt{i}", name=f"kt{i}") for i in range(NPT)]
        v_sb = [actp.tile([128, HPC * VW], bf16, tag=f"v{i}", name=f"v{i}") for i in range(NST)]
        ctxT_sb = [actp.tile([128, S], bf16, tag=f"ctxT{i}", name=f"ctxT{i}") for i in range(NPT)]

        # ---- PSUM pools (8 banks: sc 2x2 + cx 1x2 + mm 2x1) ----
        ps_sc = ctx.enter_context(tc.tile_pool(name="ps_sc", bufs=2, space="PSUM"))
        ps_cx = ctx.enter_context(tc.tile_pool(name="ps_cx", bufs=1, space="PSUM"))
        ps_mm = ctx.enter_context(tc.tile_pool(name="ps_mm", bufs=2, space="PSUM"))
        esp = ctx.enter_context(tc.tile_pool(name="esp", bufs=18))
        pop = ctx.enter_context(tc.tile_pool(name="pop", bufs=12))
        rbp = ctx.enter_context(tc.tile_pool(name="rbp", bufs=3))
        smal = ctx.enter_context(tc.tile_pool(name="smal", bufs=2))
        osb = ctx.enter_context(tc.tile_pool(name="osb", bufs=3))

        def emit_qk_proj(pt):
            for jq in range(NQB):
                sl = bass.ts(jq, 512)
                for dst, w, bias in ((qt_sb, wq_sb, bq_sb), (kt_sb, wk_sb, bk_sb)):
                    ps = ps_mm.tile([128, 512], f32, tag="mm", name="ps_p")
                    for kb in range(NKB):
                        nc.tensor.matmul(
                            ps[:],
                            w[:, kb * DHC + pt * 128:kb * DHC + (pt + 1) * 128],
                            hT_sb[:, kb * S + jq * 512:kb * S + jq * 512 + 512],
                            start=(kb == 0), stop=(kb == NKB - 1),
                        )
                    nc.vector.tensor_scalar_add(dst[pt][:, sl], ps[:], bias[:, pt:pt + 1])

        V_PASSES = {0: (0, 6)}  # head h -> head-range to project

        def emit_v_proj(st, h0, h1):
            d0, d1 = h0 * D, h1 * D
            ps = ps_mm.tile([128, DHC], f32, tag="mm", name="ps_v")
            for kb in range(NKB):
                nc.tensor.matmul(
                    ps[:, 0:d1 - d0],
                    hT_sb[:, kb * S + st * 128:kb * S + (st + 1) * 128],
                    wv_sb[:, kb * DHC + d0:kb * DHC + d1],
                    start=(kb == 0), stop=False,
                )
            nc.tensor.matmul(
                ps[:, 0:d1 - d0], hT_one[:, bass.ts(st, 128)], wv_one[:, d0:d1],
                start=False, stop=True,
            )
            # scatter into per-head 65-wide blocks; col 64 of each = 1.0
            # (memset only the ones columns: a full-tile memset would add a
            # same-engine WAW wait that the TS ISA struct can't encode)
            for hh in range(h0, h1):
                nc.vector.memset(v_sb[st][:, hh * VW + D:hh * VW + VW], 1.0)
            for hh in range(h0, h1):
                # tensor_scalar_add as copy: DVE TensorCopy lowers to a
                # 1-sync-wait ISA struct that Tile can overrun
                nc.vector.tensor_scalar_add(
                    v_sb[st][:, hh * VW:hh * VW + D],
                    ps[:, (hh - h0) * D:(hh - h0 + 1) * D],
                    0.0,
                )

        # ---- attention: per head, PV(kt) chases exp(kt); head-0 V-projection
        # tiles fill the PE stalls while ACT works; normalize chain of head h
        # overlaps head h+1 scores on DVE/Pool/DMA ----
        for h in range(HPC):
            pt, rb = h // 2, (h % 2) * D
            if h == 0:
                emit_qk_proj(0)
            cx = ps_cx.tile([VW, S], f32, tag="cx", name="cx")
            es = []
            for kt in range(NST):
                ksl = bass.ts(kt, 128)
                ps = ps_sc.tile([128, S], f32, tag="sc", name="ps_s")
                for jq in range(NQB):
                    sl = bass.ts(jq, 512)
                    nc.tensor.matmul(
                        ps[:, sl],
                        kt_sb[pt][rb:rb + D, ksl],
                        qt_sb[pt][rb:rb + D, sl],
                        start=True, stop=True,
                    )
                e = esp.tile([128, S], bf16, tag="es", name="es")
                nc.scalar.activation(e[:], ps[:], Exp, scale=0.125)
                es.append(e)
                if h in V_PASSES:
                    emit_v_proj(kt, *V_PASSES[h])
                for jq in range(NQB):
                    sl = bass.ts(jq, 512)
                    nc.tensor.matmul(
                        cx[:, sl],
                        v_sb[kt][:, h * VW:(h + 1) * VW],
                        e[:, sl],
                        start=(kt == 0), stop=(kt == NST - 1),
                    )
            rc = smal.tile([1, S], bf16, tag="rc", name="rc")
            nc.vector.reciprocal(rc[:], cx[D:VW, :])
            rbt = rbp.tile([128, S], bf16, tag="rb", name="rbt")
            nc.gpsimd.partition_broadcast(rbt[:], rc[:])
            nc.vector.tensor_mul(
                ctxT_sb[pt][rb:rb + D, :], cx[0:D, :], rbt[0:D, :]
            )
            for kt in range(NST):
                po = pop.tile([128, S], bf16, tag="po", name="po")
                eng = nc.gpsimd if kt in (2, 5) else nc.vector
                eng.tensor_mul(po[:], es[kt][:], rbt[:])
                deng = nc.gpsimd if kt % 2 else nc.sync
                deng.dma_start(probT[h, kt * 128:(kt + 1) * 128, :], po[:])
            if h == 1:
                emit_qk_proj(1)
            elif h == 3:
                emit_qk_proj(2)

        # ---- out projection tail: PSUM-accumulated, ACT evictions ----
        for ot in range(HID // 128):
            osl = bass.ts(ot, 128)
            for jq in range(NQB):
                sl = bass.ts(jq, 512)
                ps = ps_mm.tile([128, 512], f32, tag="mm", name="ps_o")
                for kb in range(NPT):
                    nc.tensor.matmul(
                        ps[:],
                        wo_sb[:, kb * HID + ot * 128:kb * HID + (ot + 1) * 128],
                        ctxT_sb[kb][:, sl],
                        start=(kb == 0), stop=(kb == NPT - 1),
                    )
                ob = osb.tile([128, 512], bf16, tag="ob", name="ob")
                nc.scalar.copy(ob[:], ps[:])
                nc.sync.dma_start(outT[osl, sl], ob[:])

    nc.compile()
    return nc


def _get_nc():
    global _CACHED_NC
    if _CACHED_NC is None:
        _CACHED_NC = _build_nc()
    return _CACHED_NC


def _prep_in_maps(hidden_states, Wq, bq, Wk, bk, Wv, bv, Wo, bo):
    import ml_dtypes

    bf = ml_dtypes.bfloat16
    f = np.float32
    in_maps = []
    for c in range(8):
        b, half = c // 2, c % 2
        hsl = slice(half * DHC, (half + 1) * DHC)
        hTa = np.empty((HID + 1, S), bf)
        hTa[:HID] = hidden_states[b].T.astype(bf)
        hTa[HID] = 1.0
        wvTa = np.empty((HID + 1, DHC), bf)
        wvTa[:HID] = Wv[hsl, :].T.astype(bf)
        wvTa[HID] = bv[hsl].astype(bf)
        in_maps.append({
            "hT": np.ascontiguousarray(hTa),
            "wqT": np.ascontiguousarray(Wq[hsl, :].T.astype(bf)),
            "wkT": np.ascontiguousarray(Wk[hsl, :].T.astype(bf)),
            "wvT": np.ascontiguousarray(wvTa),
            "woT": np.ascontiguousarray(Wo[:, hsl].T.astype(bf)),
            "bqT": np.ascontiguousarray(bq[hsl][:, None].astype(f)),
            "bkT": np.ascontiguousarray(bk[hsl][:, None].astype(f)),
        })
    return in_maps


def _run(inputs, trace=False):
    from concourse.bass_utils import run_bass_kernel_spmd

    inputs = {k: np.asarray(v, dtype=np.float32) for k, v in inputs.items()}
    nc = _get_nc()
    in_maps = _prep_in_maps(**inputs)
    res = run_bass_kernel_spmd(nc, in_maps, core_ids=list(range(8)), trace=trace)

    bo = inputs["bo"]
    out = np.empty((B, S, HID), np.float32)
    probs = np.empty((B, NH, S, S), np.float32)
    for c in range(8):
        b, half = c // 2, c % 2
        r = res.results[c]
        probs[b, half * HPC:(half + 1) * HPC] = r["probT"].transpose(0, 2, 1).astype(np.float32)
        if half == 0:
            out[b] = r["outT"].T.astype(np.float32)
        else:
            out[b] += r["outT"].T.astype(np.float32)
    out += bo
    return (out, probs), res


def kernel(**inputs):
    return _run(inputs, trace=False)[0]


# revision 56
# speedup vs baseline: 1.0407x; 1.0407x over previous
"""MHA forward (B=4, S=1024, HID=768, NH=12, D=64) on 8 NeuronCores.

Sharding: core c -> batch b = c//2, head half = c%2 (6 heads each).
All on-device tensors live transposed ([feature, seq]) so every matmul
contraction dim lands on partitions with no on-device transposes:
  QT/KT [dh, s] <- WxT-slice^T @ hiddenT;  V [s, dh] <- hiddenT^T @ WvT
  ST [k, q] <- KT-slice^T @ QT-slice       (scores, no max-subtraction:
                                            |scores/8| < ~2 for this input dist)
  expS = exp(ST/8) via ACT (PSUM -> SBUF, bf16)
  ctxT [dh, q] <- V_aug^T @ expS, where V_aug has a ones column per head
                  -> row 64 of the PSUM tile is the softmax denominator
  probsT = expS * bcast(1/denom)  (bcast via GPSIMD partition_broadcast)
  outT [ho, s] <- WoT-slice^T @ ctxT
Host: converts weights/hidden to bf16, transposes probsT back (stored bf16,
upcast on host), sums the two per-batch outT partials, adds bo. PE runs bf16
with fp32 PSUM accumulation; heads are software-pipelined so the normalize
chain of head h overlaps head h+1 scores on DVE/Pool/DMA.
"""

import numpy as np

B, S, HID, NH, D = 4, 1024, 768, 12, 64
HPC = NH // 2          # heads per core = 6
DHC = HPC * D          # dh per core = 384
VW = D + 1             # V block width incl. ones column = 65

_CACHED_NC = None


def _build_nc():
    from contextlib import ExitStack

    import concourse.bass as bass
    import concourse.mybir as mybir
    import concourse.tile as tile
    from concourse import bacc

    f32 = mybir.dt.float32
    bf16 = mybir.dt.bfloat16
    Exp = mybir.ActivationFunctionType.Exp

    nc = bacc.Bacc("TRN2", target_bir_lowering=False, debug=False, num_devices=8)
    hT = nc.dram_tensor("hT", [HID + 1, S], bf16, kind="ExternalInput")
    wqT = nc.dram_tensor("wqT", [HID, DHC], bf16, kind="ExternalInput")
    wkT = nc.dram_tensor("wkT", [HID, DHC], bf16, kind="ExternalInput")
    wvT = nc.dram_tensor("wvT", [HID + 1, DHC], bf16, kind="ExternalInput")
    woT = nc.dram_tensor("woT", [DHC, HID], bf16, kind="ExternalInput")
    bqT = nc.dram_tensor("bqT", [DHC, 1], f32, kind="ExternalInput")
    bkT = nc.dram_tensor("bkT", [DHC, 1], f32, kind="ExternalInput")
    probT = nc.dram_tensor("probT", [HPC, S, S], bf16, kind="ExternalOutput")
    outT = nc.dram_tensor("outT", [HID, S], bf16, kind="ExternalOutput")

    NKB = HID // 128     # 6 contraction blocks for projections
    NPT = DHC // 128     # 3 partition tiles for QT/KT/ctxT
    NST = S // 128       # 8 seq tiles
    NQB = S // 512       # 2 moving blocks

    with (
        nc.allow_low_precision(reason="bf16 matmul staging; fp32 accumulate"),
        tile.TileContext(nc) as tc,
        ExitStack() as ctx,
    ):
        const = ctx.enter_context(tc.tile_pool(name="const", bufs=1))
        wpool = ctx.enter_context(tc.tile_pool(name="w", bufs=1))
        actp = ctx.enter_context(tc.tile_pool(name="acts", bufs=1))

        # ---- loads (consolidated: one DMA per tensor via rearranged APs) ----
        hT_sb = wpool.tile([128, NKB * S], bf16, tag="hT", name="hT_sb")
        wq_sb = wpool.tile([128, NKB * DHC], bf16, tag="wq", name="wq_sb")
        wk_sb = wpool.tile([128, NKB * DHC], bf16, tag="wk", name="wk_sb")
        wv_sb = wpool.tile([128, NKB * DHC], bf16, tag="wv", name="wv_sb")
        wo_sb = wpool.tile([128, NPT * HID], bf16, tag="wo", name="wo_sb")
        hT_one = wpool.tile([1, S], bf16, tag="hT_one")
        wv_one = wpool.tile([1, DHC], bf16, tag="wv_one")
        bq_sb = wpool.tile([128, NPT], f32, tag="bq", name="bq_sb")
        bk_sb = wpool.tile([128, NPT], f32, tag="bk", name="bk_sb")

        wq3 = wq_sb[:].rearrange("p (t d) -> p t d", t=NKB)
        wk3 = wk_sb[:].rearrange("p (t d) -> p t d", t=NKB)
        hT3 = hT_sb[:].rearrange("p (t s) -> p t s", t=NKB)
        wqT3 = wqT[:].rearrange("(t p) d -> p t d", p=128)
        wkT3 = wkT[:].rearrange("(t p) d -> p t d", p=128)
        hTT3 = hT[0:HID, :].rearrange("(t p) s -> p t s", p=128)
        for c in range(3):
            ksl = slice(2 * c, 2 * c + 2)
            nc.sync.dma_start(wq3[:, ksl], wqT3[:, ksl])
            nc.sync.dma_start(wk3[:, ksl], wkT3[:, ksl])
            nc.sync.dma_start(hT3[:, ksl], hTT3[:, ksl])
        nc.sync.dma_start(
            bq_sb[:].rearrange("p (t o) -> p t o", o=1),
            bqT[:].rearrange("(t p) o -> p t o", p=128),
        )
        nc.sync.dma_start(
            bk_sb[:].rearrange("p (t o) -> p t o", o=1),
            bkT[:].rearrange("(t p) o -> p t o", p=128),
        )
        nc.sync.dma_start(hT_one[:], hT[HID:HID + 1, :])
        nc.sync.dma_start(
            wv_sb[:].rearrange("p (t d) -> p t d", t=NKB),
            wvT[0:HID, :].rearrange("(t p) d -> p t d", p=128),
        )
        nc.sync.dma_start(wv_one[:], wvT[HID:HID + 1, :])
        nc.sync.dma_start(
            wo_sb[:].rearrange("p (t d) -> p t d", t=NPT),
            woT[:].rearrange("(t p) d -> p t d", p=128),
        )

        ones_row = const.tile([1, 128], bf16, tag="ones_row")
        nc.vector.memset(ones_row[:], 1.0)

        qt_sb = [actp.tile([128, S], bf16, tag=f"qt{i}", name=f"qt{i}") for i in range(NPT)]
        kt_sb = [actp.tile([128, S], bf16, tag=f"kt{i}", name=f"kt{i}") for i in range(NPT)]
        v_sb = [actp.tile([128, HPC * VW], bf16, tag=f"v{i}", name=f"v{i}") for i in range(NST)]
        ctxT_sb = [actp.tile([128, S], bf16, tag=f"ctxT{i}", name=f"ctxT{i}") for i in range(NPT)]

        # ---- PSUM pools (8 banks: sc 2x2 + cx 1x2 + mm 2x1) ----
        ps_sc = ctx.enter_context(tc.tile_pool(name="ps_sc", bufs=2, space="PSUM"))
        ps_cx = ctx.enter_context(tc.tile_pool(name="ps_cx", bufs=1, space="PSUM"))
        ps_mm = ctx.enter_context(tc.tile_pool(name="ps_mm", bufs=2, space="PSUM"))
        esp = ctx.enter_context(tc.tile_pool(name="esp", bufs=18))
        pop = ctx.enter_context(tc.tile_pool(name="pop", bufs=12))
        rbp = ctx.enter_context(tc.tile_pool(name="rbp", bufs=3))
        smal = ctx.enter_context(tc.tile_pool(name="smal", bufs=2))
        osb = ctx.enter_context(tc.tile_pool(name="osb", bufs=12))

        def emit_qk_proj(pt):
            for jq in range(NQB):
                sl = bass.ts(jq, 512)
                for dst, w, bias in ((qt_sb, wq_sb, bq_sb), (kt_sb, wk_sb, bk_sb)):
                    ps = ps_mm.tile([128, 512], f32, tag="mm", name="ps_p")
                    for kb in range(NKB):
                        nc.tensor.matmul(
                            ps[:],
                            w[:, kb * DHC + pt * 128:kb * DHC + (pt + 1) * 128],
                            hT_sb[:, kb * S + jq * 512:kb * S + jq * 512 + 512],
                            start=(kb == 0), stop=(kb == NKB - 1),
                        )
                    nc.vector.tensor_scalar_add(dst[pt][:, sl], ps[:], bias[:, pt:pt + 1])

        V_PASSES = {0: (0, 6)}  # head h -> head-range to project

        def emit_v_proj(st, h0, h1):
            d0, d1 = h0 * D, h1 * D
            ps = ps_mm.tile([128, DHC], f32, tag="mm", name="ps_v")
            for kb in range(NKB):
                nc.tensor.matmul(
                    ps[:, 0:d1 - d0],
                    hT_sb[:, kb * S + st * 128:kb * S + (st + 1) * 128],
                    wv_sb[:, kb * DHC + d0:kb * DHC + d1],
                    start=(kb == 0), stop=False,
                )
            nc.tensor.matmul(
                ps[:, 0:d1 - d0], hT_one[:, bass.ts(st, 128)], wv_one[:, d0:d1],
                start=False, stop=True,
            )
            # scatter into per-head 65-wide blocks; col 64 of each = 1.0
            # (memset only the ones columns: a full-tile memset would add a
            # same-engine WAW wait that the TS ISA struct can't encode)
            for hh in range(h0, h1):
                nc.vector.memset(v_sb[st][:, hh * VW + D:hh * VW + VW], 1.0)
            for hh in range(h0, h1):
                # tensor_scalar_add as copy: DVE TensorCopy lowers to a
                # 1-sync-wait ISA struct that Tile can overrun
                nc.vector.tensor_scalar_add(
                    v_sb[st][:, hh * VW:hh * VW + D],
                    ps[:, (hh - h0) * D:(hh - h0 + 1) * D],
                    0.0,
                )

        # ---- attention: per head, PV(kt) chases exp(kt); head-0 V-projection
        # tiles fill the PE stalls while ACT works; normalize chain of head h
        # overlaps head h+1 scores on DVE/Pool/DMA ----
        for h in range(HPC):
            pt, rb = h // 2, (h % 2) * D
            if h == 0:
                emit_qk_proj(0)
            cx = ps_cx.tile([VW, S], f32, tag="cx", name="cx")
            es = []
            for kt in range(NST):
                ksl = bass.ts(kt, 128)
                ps = ps_sc.tile([128, S], f32, tag="sc", name="ps_s")
                for jq in range(NQB):
                    sl = bass.ts(jq, 512)
                    nc.tensor.matmul(
                        ps[:, sl],
                        kt_sb[pt][rb:rb + D, ksl],
                        qt_sb[pt][rb:rb + D, sl],
                        start=True, stop=True,
                    )
                e = esp.tile([128, S], bf16, tag="es", name="es")
                nc.scalar.activation(e[:], ps[:], Exp, scale=0.125)
                es.append(e)
                if h in V_PASSES:
                    emit_v_proj(kt, *V_PASSES[h])
                for jq in range(NQB):
                    sl = bass.ts(jq, 512)
                    nc.tensor.matmul(
                        cx[:, sl],
                        v_sb[kt][:, h * VW:(h + 1) * VW],
                        e[:, sl],
                        start=(kt == 0), stop=(kt == NST - 1),
                    )
            rc = smal.tile([1, S], bf16, tag="rc", name="rc")
            nc.vector.reciprocal(rc[:], cx[D:VW, :])
            rbt = rbp.tile([128, S], bf16, tag="rb", name="rbt")
            nc.gpsimd.partition_broadcast(rbt[:], rc[:])
            nc.vector.tensor_mul(
                ctxT_sb[pt][rb:rb + D, :], cx[0:D, :], rbt[0:D, :]
            )
            for kt in range(NST):
                po = pop.tile([128, S], bf16, tag="po", name="po")
                eng = nc.gpsimd if kt in (2, 5) else nc.vector
                eng.tensor_mul(po[:], es[kt][:], rbt[:])
                deng = nc.gpsimd if kt % 2 else nc.sync
                deng.dma_start(probT[h, kt * 128:(kt + 1) * 128, :], po[:])
            if h == 1:
                emit_qk_proj(1)
            elif h == 3:
                emit_qk_proj(2)

        # ---- out projection tail: PSUM-accumulated, ACT evictions ----
        for ot in range(HID // 128):
            osl = bass.ts(ot, 128)
            for jq in range(NQB):
                sl = bass.ts(jq, 512)
                ps = ps_mm.tile([128, 512], f32, tag="mm", name="ps_o")
                for kb in range(NPT):
                    nc.tensor.matmul(
                        ps[:],
                        wo_sb[:, kb * HID + ot * 128:kb * HID + (ot + 1) * 128],
                        ctxT_sb[kb][:, sl],
                        start=(kb == 0), stop=(kb == NPT - 1),
                    )
                ob = osb.tile([128, 512], bf16, tag="ob", name="ob")
                nc.scalar.copy(ob[:], ps[:])
                nc.sync.dma_start(outT[osl, sl], ob[:])

    nc.compile()
    return nc


def _get_nc():
    global _CACHED_NC
    if _CACHED_NC is None:
        _CACHED_NC = _build_nc()
    return _CACHED_NC


def _prep_in_maps(hidden_states, Wq, bq, Wk, bk, Wv, bv, Wo, bo):
    import ml_dtypes

    bf = ml_dtypes.bfloat16
    f = np.float32
    in_maps = []
    for c in range(8):
        b, half = c // 2, c % 2
        hsl = slice(half * DHC, (half + 1) * DHC)
        hTa = np.empty((HID + 1, S), bf)
        hTa[:HID] = hidden_states[b].T.astype(bf)
        hTa[HID] = 1.0
        wvTa = np.empty((HID + 1, DHC), bf)
        wvTa[:HID] = Wv[hsl, :].T.astype(bf)
        wvTa[HID] = bv[hsl].astype(bf)
        in_maps.append({
            "hT": np.ascontiguousarray(hTa),
            "wqT": np.ascontiguousarray(Wq[hsl, :].T.astype(bf)),
            "wkT": np.ascontiguousarray(Wk[hsl, :].T.astype(bf)),
            "wvT": np.ascontiguousarray(wvTa),
            "woT": np.ascontiguousarray(Wo[:, hsl].T.astype(bf)),
            "bqT": np.ascontiguousarray(bq[hsl][:, None].astype(f)),
            "bkT": np.ascontiguousarray(bk[hsl][:, None].astype(f)),
        })
    return in_maps


def _run(inputs, trace=False):
    from concourse.bass_utils import run_bass_kernel_spmd

    inputs = {k: np.asarray(v, dtype=np.float32) for k, v in inputs.items()}
    nc = _get_nc()
    in_maps = _prep_in_maps(**inputs)
    res = run_bass_kernel_spmd(nc, in_maps, core_ids=list(range(8)), trace=trace)

    bo = inputs["bo"]
    out = np.empty((B, S, HID), np.float32)
    probs = np.empty((B, NH, S, S), np.float32)
    for c in range(8):
        b, half = c // 2, c % 2
        r = res.results[c]
        probs[b, half * HPC:(half + 1) * HPC] = r["probT"].transpose(0, 2, 1).astype(np.float32)
        if half == 0:
            out[b] = r["outT"].T.astype(np.float32)
        else:
            out[b] += r["outT"].T.astype(np.float32)
    out += bo
    return (out, probs), res


def kernel(**inputs):
    return _run(inputs, trace=False)[0]
